# revision 15
# baseline (speedup 1.0000x reference)
"""Stage 1.5: baseline bf16 conv + engine rebalance (paired sign, PReLU from
PSUM on ACT, 2-port TT-max pool + ts/stt accum stats on DVE), bf16 input,
fp16 output. No fp8, no tensor_tensor_reduce."""

import uuid

import numpy as np
import ml_dtypes
import jax

jax.config.update("jax_enable_compilation_cache", False)

import concourse.bacc as bacc
import concourse.mybir as mybir
import concourse.tile as tile
from concourse.bass_utils import run_bass_kernel_spmd

AF = mybir.ActivationFunctionType
ALU = mybir.AluOpType

N_CORES = 8
N = 128
NB = N // N_CORES
CI = 64
CO = 128
L = 4096
LP = L + 7
LPA = L + 8
LO = L // 2
K = 7
PAD_VAL = -1.0
EPS = 1e-5
M_GLOBAL = float(N * LO)

XT_BUFS = 3
ST_BUFS = 2
AT_BUFS = 6


def _build(alpha: float):
    nc = bacc.Bacc("TRN2", target_bir_lowering=False, debug=False,
                   num_devices=N_CORES)

    xs = nc.dram_tensor("xs", [NB * CI, L], mybir.dt.bfloat16, kind="ExternalInput")
    wts = nc.dram_tensor("wts", [128, 8 * 128], mybir.dt.bfloat16, kind="ExternalInput")
    gb = nc.dram_tensor("gb", [128, 2], mybir.dt.float32, kind="ExternalInput")
    out = nc.dram_tensor("out", [NB * CO, LO], mybir.dt.float16, kind="ExternalOutput")

    with tile.TileContext(nc) as tc:
        with (
            tc.tile_pool(name="wp", bufs=1) as wp,
            tc.tile_pool(name="xp", bufs=XT_BUFS) as xp,
            tc.tile_pool(name="stp", bufs=ST_BUFS) as stp,
            tc.tile_pool(name="ap", bufs=AT_BUFS) as ap_pool,
            tc.tile_pool(name="pp", bufs=2, space="PSUM") as pp,
            tc.tile_pool(name="zp", bufs=3) as zp,
            tc.tile_pool(name="qp", bufs=2) as qp,
            tc.tile_pool(name="yp", bufs=NB) as yp,
            tc.tile_pool(name="sp", bufs=1) as sp,
            tc.tile_pool(name="op", bufs=3) as op_pool,
        ):
            wt = wp.tile([128, 8 * 128], mybir.dt.bfloat16)
            nc.scalar.dma_start(wt[:], wts[:])
            gbt = wp.tile([128, 2], mybir.dt.float32)
            nc.scalar.dma_start(gbt[:], gb[:])

            warm = wp.tile([128, 2], mybir.dt.float32)
            nc.vector.memset(warm[:], 1.0)
            nc.scalar.activation(warm[:, 0:1], warm[:, 0:1], AF.Sign)
            nc.scalar.activation(warm[:, 1:2], warm[:, 1:2], AF.Prelu, alpha=alpha)

            _build_pass(nc, tc, xs, out, wt, gbt, alpha,
                        xp, stp, ap_pool, pp, zp, qp, yp, sp, op_pool)

    nc.compile()
    nc.m.name = f"bk{uuid.uuid4().hex[:10]}"
    return nc


def _build_pass(nc, tc, xs, out, wt, gbt, alpha,
                xp, stp, ap_pool, pp, zp, qp, yp, sp, op_pool):
    stats = sp.tile([128, 32], mybir.dt.float32, name="stats", tag="stats")

    y_tiles = []
    for bp in range(NB // 2):
        xt = xp.tile([128, LPA], mybir.dt.bfloat16, name=f"xt{bp}", tag="xt")
        if bp < XT_BUFS:
            # pads persist across buffer reuse: the x DMA only writes 3:4099
            nc.gpsimd.memset(xt[:, 0:3], PAD_VAL)
            nc.gpsimd.memset(xt[:, L + 3:LPA], PAD_VAL)
        nc.sync.dma_start(xt[:, 3:L + 3], xs[bp * 128:(bp + 1) * 128, :])

        # one Sign op covers both batches of the pair (ACT cost scales with
        # free dim only); copies build the per-batch baseline at layouts
        st = stp.tile([128, LPA], mybir.dt.bfloat16, name=f"st{bp}", tag="st")
        nc.scalar.activation(st[:], xt[:], AF.Sign)

        # even batch: rows 0:64 direct, 64:128 shifted-by-1 (even weights)
        # odd batch:  rows 0:64 shifted-by-1, 64:128 direct (odd weights)
        at0 = ap_pool.tile([128, LPA], mybir.dt.bfloat16, name=f"at{2*bp}", tag="at")
        at1 = ap_pool.tile([128, LPA], mybir.dt.bfloat16, name=f"at{2*bp+1}", tag="at")
        nc.sync.dma_start(at0[0:64, 0:LPA], st[0:64, 0:LPA])
        nc.sync.dma_start(at0[64:128, 0:LPA - 2], st[0:64, 1:LPA - 1])
        nc.sync.dma_start(at1[0:64, 0:LPA - 2], st[64:128, 1:LPA - 1])
        nc.sync.dma_start(at1[64:128, 0:LPA], st[64:128, 0:LPA])

        for sub in range(2):
            b = 2 * bp + sub
            at = at0 if sub == 0 else at1

            yt = yp.tile([128, LO], mybir.dt.float16, name=f"yt{b}", tag="yt")
            y_tiles.append(yt)

            for half in range(2):
                h = half * 2048
                ps = pp.tile([128, 2048], mybir.dt.float32,
                             name=f"ps{b}_{half}", tag="ps")
                for j in range(4):
                    w_j = wt[:, (sub * 4 + j) * 128:(sub * 4 + j + 1) * 128]
                    for t in range(4):
                        nc.tensor.matmul(
                            ps[:, t * 512:(t + 1) * 512],
                            w_j,
                            at[:, h + t * 512 + 2 * j: h + t * 512 + 2 * j + 512],
                            start=(j == 0), stop=(j == 3),
                        )
                # PReLU straight out of PSUM on the 1.2GHz ACT engine
                # (commutes with the maxpool since alpha >= 0)
                z = zp.tile([128, 2048], mybir.dt.float16,
                            name=f"z{b}_{half}", tag="z")
                nc.scalar.activation(z[:], ps[:], AF.Prelu, alpha=alpha)

                # maxpool k2s2: 2-port elementwise max of strided views
                z3 = z.rearrange("p (l two) -> p two l", two=2)
                nc.vector.tensor_tensor(
                    yt[:, half * 1024:(half + 1) * 1024],
                    z3[:, 0, :], z3[:, 1, :], op=ALU.max)

            scr = qp.tile([128, LO], mybir.dt.float16, name=f"scr{b}", tag="scr")
            nc.vector.tensor_scalar(
                scr[:], yt[:], 1.0, 0.0, op0=ALU.mult, op1=ALU.add,
                accum_out=stats[:, b:b + 1])
            nc.vector.scalar_tensor_tensor(
                scr[:], yt[:], 1.0, yt[:], op0=ALU.mult, op1=ALU.mult,
                accum_out=stats[:, 16 + b:17 + b])

    # ---- allreduce + scale/shift (identical to the proven baseline) ----
    loc = sp.tile([128, 2], mybir.dt.float32, name="loc", tag="loc")
    nc.vector.tensor_reduce(loc[:, 0:1], stats[:, 0:16],
                            axis=mybir.AxisListType.X, op=ALU.add)
    nc.vector.tensor_reduce(loc[:, 1:2], stats[:, 16:32],
                            axis=mybir.AxisListType.X, op=ALU.add)

    xbuf = sp.tile([128, 16], mybir.dt.float32, name="xbuf", tag="xbuf")
    nc.vector.tensor_copy(xbuf[:, 0:2], loc[:])
    g = sp.tile([128, 2], mybir.dt.float32, name="g", tag="g")

    rsem = nc.alloc_semaphore("ar_remote")
    psem = nc.alloc_semaphore("ar_prep")
    lsem = nc.alloc_semaphore("ar_local")
    with tc.tile_critical(no_gpsimd_drain=True):
        nc.gpsimd.bir_kernel_barrier_wait([list(range(N_CORES))])
        for delta in range(1, 8):
            rd = [None] * 8
            rd[delta] = (0, delta)
            nc.gpsimd.remote_dma_broadcast(
                xbuf[:, 2 * delta:2 * delta + 2], loc[:, 0:2],
                rsem, lsem, rdests=rd,
            ).then_inc(psem, 1)
        nc.gpsimd.wait_ge(psem, 7)
        # count=None defers the doorbell to the SWDGE drain (~43us later,
        # measured); an explicit count fires the 7 broadcast preps immediately
        nc.gpsimd.trigger_dma(count=7)
        nc.vector.wait_ge(rsem, 14)
        nc.vector.tensor_reduce(
            g[:, 0:1], xbuf.rearrange("p (s two) -> p two s", two=2)[:, 0:1, :],
            axis=mybir.AxisListType.X, op=ALU.add)
        nc.vector.tensor_reduce(
            g[:, 1:2], xbuf.rearrange("p (s two) -> p two s", two=2)[:, 1:2, :],
            axis=mybir.AxisListType.X, op=ALU.add)

    v = sp.tile([128, 8], mybir.dt.float32, name="v", tag="v")
    mean, msq_eps, vareps, std, rec, t1, s_col, t_col = (
        v[:, i:i + 1] for i in range(8))
    nc.vector.tensor_scalar(mean, g[:, 0:1], 1.0 / M_GLOBAL, None, op0=ALU.mult)
    nc.vector.tensor_scalar(msq_eps, mean, mean, EPS, op0=ALU.mult, op1=ALU.subtract)
    nc.vector.scalar_tensor_tensor(
        vareps, g[:, 1:2], 1.0 / M_GLOBAL, msq_eps,
        op0=ALU.mult, op1=ALU.subtract)
    nc.scalar.activation(std, vareps, AF.Sqrt)
    nc.vector.reciprocal(rec, std)
    nc.vector.tensor_scalar(t1, rec, vareps, 0.5, op0=ALU.mult, op1=ALU.mult)
    nc.vector.scalar_tensor_tensor(std, std, 0.5, t1,
                                   op0=ALU.mult, op1=ALU.add)
    nc.vector.reciprocal(rec, std)
    nc.vector.tensor_scalar(s_col, rec, gbt[:, 0:1], None, op0=ALU.mult)
    nc.vector.tensor_scalar(t1, mean, -1.0, None, op0=ALU.mult)
    nc.vector.scalar_tensor_tensor(
        t_col, s_col, t1, gbt[:, 1:2], op0=ALU.mult, op1=ALU.add)

    for bp in range(NB // 2):
        ot = op_pool.tile([128, 2 * LO], mybir.dt.float16, name=f"ot{bp}", tag="ot")
        for sub in range(2):
            nc.vector.tensor_scalar(
                ot[:, sub * LO:(sub + 1) * LO], y_tiles[2 * bp + sub][:],
                s_col, t_col, op0=ALU.mult, op1=ALU.add)
        eng = nc.sync if bp % 2 == 0 else nc.scalar
        eng.dma_start(
            out.rearrange("(a p) l -> p a l", p=128)[:, 2 * bp:2 * bp + 2, :],
            ot.rearrange("p (a l) -> p a l", a=2))


def _prep_weights(W: np.ndarray) -> np.ndarray:
    bw = np.sign(W).astype(np.float32)  # [CO, CI, K]
    wts = np.zeros((128, 8, 128), dtype=np.float32)
    for j in range(4):
        wts[0:64, j, :] = bw[:, :, 2 * j].T
        if 2 * j + 1 < K:
            wts[64:128, j, :] = bw[:, :, 2 * j + 1].T
        if 2 * j + 1 < K:
            wts[0:64, 4 + j, :] = bw[:, :, 2 * j + 1].T
        wts[64:128, 4 + j, :] = bw[:, :, 2 * j].T
    return wts.reshape(128, 8 * 128).astype(ml_dtypes.bfloat16)


def _make_in_maps(x, W, gamma, beta):
    wts = _prep_weights(np.asarray(W))
    gb = np.stack([np.asarray(gamma, dtype=np.float32),
                   np.asarray(beta, dtype=np.float32)], axis=1)
    xb = np.asarray(x).astype(ml_dtypes.bfloat16)
    in_maps = []
    for c in range(N_CORES):
        shard = np.ascontiguousarray(
            xb[c * NB:(c + 1) * NB].reshape(NB * CI, L))
        in_maps.append({"xs": shard, "wts": wts, "gb": gb})
    return in_maps


_NC_CACHE = {}


def kernel(x, W, prelu_w, gamma, beta):
    x = np.asarray(x)
    alpha = float(np.asarray(prelu_w).reshape(-1)[0])
    assert alpha >= 0.0, "prelu-before-pool rewrite needs alpha >= 0"
    assert x.shape == (N, CI, L), x.shape

    key = alpha
    if key not in _NC_CACHE:
        _NC_CACHE[key] = _build(alpha)
    nc = _NC_CACHE[key]

    in_maps = _make_in_maps(x, W, gamma, beta)
    res = run_bass_kernel_spmd(nc, in_maps, core_ids=list(range(N_CORES)))
    outs = [res.results[c]["out"].reshape(NB, CO, LO).astype(np.float32)
            for c in range(N_CORES)]
    return np.concatenate(outs, axis=0)
